# revision 30
# baseline (speedup 1.0000x reference)
"""ArrowLoRA MoE routing kernel for 8 TRN2 NeuronCores.

Math (per token t of 8192, F=2048, E=16 experts, R=16, O=2048):
    sim   = |x @ protos.T|; coeff = softmax(top4-masked sim)
    z     = x @ Acat.T;     delta = scaling * (coeff * z per expert) @ Bcat

Token-parallel across 8 cores (1024 tokens each), weights replicated, no
collectives. Host pre-transposes/casts x to fp16 (bf16 flips ~0.7% of
top-4 picks and fails the 2e-2 gate; fp16 misroutes ~0.05% -> rel_err
~7.4e-3 incl. fp16 output). Raw Bass engine streams with hand-placed,
transitively-minimal semaphores; x is DMAed in (group, token-half)
pieces so tiles 0-3 start their routing/second matmul ~6us before the
full input lands. Measured ~59-62us (vs ~73us Tile framework, 78us
first working version); remaining time is ~8us fixed NEFF start, input
DMA at the HBM cap, ~35us PE busy, and ~9us runtime end sequence.
fp8/DoubleRow mm2 was tested and REJECTED: e4m3 zw/Bcat quantization
gives rel_err 0.039 > the 2e-2 gate.

Pipeline per core (1024 tokens = 8 tiles of 128):
  PE : mm1(i) [16 accum matmuls -> zs psum]  |  transp(i-1), mm2(i-1)
  ACT: |sim| -> exp (no max-shift; |sim|<=~6) ; zwT copies ; half epilogue
  DVE: top8 -> mask -> softmax -> coeff ; zw = z*coeff (reads PSUM) ; half epilogue
  SP : input DMAs (x groups)   GpSimd: W/Bc/ident DMAs + output DMAs
"""

import os

import numpy as np

import concourse.bass as bass
import concourse.mybir as mybir
from concourse import bacc
from concourse.bass import ts
from concourse.bass_utils import run_bass_kernel_spmd

B, S, F, E, R, O = 4, 2048, 2048, 16, 16, 2048
TOPK = 4
NCORES = 8
T = B * S
TPC = T // NCORES          # 1024
NT = TPC // 128            # 8
FCH = F // 128             # 16
ER = E * R                 # 256
ERCH = ER // 128           # 2
WCOLS = ER + E             # 272
XG = 4
NG = FCH // XG

F16 = mybir.dt.float16
F32 = mybir.dt.float32
ALU = mybir.AluOpType
AF = mybir.ActivationFunctionType

_CACHE = {}
LAST_RESULTS = None


def build_nc():
    nc = bacc.Bacc(target_bir_lowering=False)

    xT = nc.declare_dram_parameter("xT", [NG, 2, 128, XG, TPC // 2], F16, isOutput=False)
    W = nc.declare_dram_parameter("W", [128, FCH, WCOLS], F16, isOutput=False)
    Bc = nc.declare_dram_parameter("Bc", [ERCH, 128, O], F16, isOutput=False)
    ident = nc.declare_dram_parameter("ident", [128, 128], F16, isOutput=False)
    out = nc.declare_dram_parameter("out", [NT // 2, 128, 2, O], F16, isOutput=True)

    # ---- SBUF ----
    x_sb = nc.alloc_sbuf_tensor("x_sb", [128, NG, 2, XG, TPC // 2], F16).ap()
    w_sb = nc.alloc_sbuf_tensor("w_sb", [128, FCH, WCOLS], F16).ap()
    bc_sb = nc.alloc_sbuf_tensor("bc_sb", [128, ERCH, O], F16).ap()
    id_sb = nc.alloc_sbuf_tensor("id_sb", [128, 128], F16).ap()
    sim_a = nc.alloc_sbuf_tensor("sim_a", [128, NT, E], F32).ap()
    m8_a = nc.alloc_sbuf_tensor("m8_a", [128, NT, 8], F32).ap()
    e_a = nc.alloc_sbuf_tensor("e_a", [128, NT, E], F32).ap()
    ge_a = nc.alloc_sbuf_tensor("ge_a", [128, NT, E], F32).ap()
    em_a = nc.alloc_sbuf_tensor("em_a", [128, NT, E], F32).ap()
    den_a = nc.alloc_sbuf_tensor("den_a", [128, NT], F32).ap()
    rcp_a = nc.alloc_sbuf_tensor("rcp_a", [128, NT], F32).ap()
    co_a = nc.alloc_sbuf_tensor("co_a", [128, NT, E], F32).ap()
    zw_sb = nc.alloc_sbuf_tensor("zw_sb", [128, 2, E, R], F16).ap()      # 2 bufs
    zwT_sb = nc.alloc_sbuf_tensor("zwT_sb", [128, 2, ERCH, 128], F16).ap()
    out_sb = nc.alloc_sbuf_tensor("out_sb", [128, 2, 2, O], F16).ap()    # 2 pairs

    # ---- PSUM: 8 banks = zs x2 + tp x2 + delta x4 ----
    zs_ps = [nc.alloc_psum_tensor(f"zs{s}", [128, WCOLS], F32).ap() for s in range(2)]
    tp_ps = [nc.alloc_psum_tensor(f"tp{s}", [128, 128], F16).ap() for s in range(2)]
    d_ps = [nc.alloc_psum_tensor(f"d{q}", [128, 512], F32).ap() for q in range(4)]

    from contextlib import ExitStack
    _sems = ExitStack()

    def sem(name):
        return _sems.enter_context(nc.semaphore(name))

    s_w, s_wr = sem("s_w"), sem("s_wr")
    s_x00, s_x10, s_x20, s_x30 = sem("s_x00"), sem("s_x10"), sem("s_x20"), sem("s_x30")
    s_x01, s_x11, s_x21, s_x31 = sem("s_x01"), sem("s_x11"), sem("s_x21"), sem("s_x31")
    s_id, s_bc = sem("s_id"), sem("s_bc")
    s_mm1, s_abs, s_exp, s_zw = sem("s_mm1"), sem("s_abs"), sem("s_exp"), sem("s_zw")
    s_tp, s_zwt, s_d = sem("s_tp"), sem("s_zwt"), sem("s_d")
    s_epA, s_epV = sem("s_epA"), sem("s_epV")
    s_out0, s_out1 = sem("s_out0"), sem("s_out1")

    with nc.Block(no_gpsimd_drain=True) as block:
        s_x = [[s_x00, s_x01], [s_x10, s_x11], [s_x20, s_x21], [s_x30, s_x31]]

        @block.sync
        def _(sp):
            # Priority order: W chunk 0 (70KB), x(group0, token-half0), rest
            # of W, remaining half-0 groups (tiles 0-3 fully covered ~6us
            # earlier than a full-x wait), then half-1 groups.
            sp.dma_start(out=w_sb[:, 0:1, :], in_=W[:, 0:1, :]).then_inc(s_w, 16)
            sp.dma_start(out=x_sb[:, 0, 0], in_=xT[0, 0]).then_inc(s_x[0][0], 16)
            sp.dma_start(out=w_sb[:, 1:FCH, :], in_=W[:, 1:FCH, :]).then_inc(s_wr, 16)
            for g in range(1, NG):
                sp.dma_start(out=x_sb[:, g, 0], in_=xT[g, 0]).then_inc(s_x[g][0], 16)
            for g in range(NG):
                sp.dma_start(out=x_sb[:, g, 1], in_=xT[g, 1]).then_inc(s_x[g][1], 16)

        @block.gpsimd
        def _(gp):
            # gpsimd starts ~6us late (library load) - only late-needed DMAs.
            # Wait for x0 so bc/ident descriptors queue behind the
            # critical-path x groups, not ahead of them.
            gp.wait_ge(s_x00, 16)
            gp.dma_start(
                out=bc_sb, in_=Bc[:, :, :].rearrange("c p o -> p c o")
            ).then_inc(s_bc, 16)
            gp.dma_start(out=id_sb, in_=ident[:, :]).then_inc(s_id, 16)
            s_out = [s_out0, s_out1]
            for k in range(NT // 2 - 1):
                gp.wait_ge(s_epA, 4 * (k + 1))
                gp.wait_ge(s_epV, 4 * (k + 1))
                gp.dma_start(out=out[k], in_=out_sb[:, k % 2]).then_inc(s_out[k % 2], 16)
            kl = NT // 2 - 1
            for half in range(2):
                gp.wait_ge(s_epA, 4 * kl + 2 * (half + 1))
                gp.wait_ge(s_epV, 4 * kl + 2 * (half + 1))
                gp.dma_start(
                    out=out[kl][:, half:half + 1, :],
                    in_=out_sb[:, kl % 2, half:half + 1, :],
                ).then_inc(s_out[kl % 2], 16)
            gp.wait_ge(s_out0, 32)
            gp.wait_ge(s_out1, 48)

        @block.tensor
        def _(te):
            def pe_transp(j):
                te.wait_ge(s_zw, j + 1)
                if j == 0:
                    te.wait_ge(s_id, 16)
                for h in range(ERCH):
                    m = ERCH * j + h
                    # tp slot WAR vs ACT copy(m-2) is covered transitively:
                    # pe_mm2(j-1) already waited s_zwt >= 2j >= m-1.
                    te.matmul(
                        tp_ps[m % 2],
                        lhsT=zw_sb[:, j % 2].rearrange("p e r -> p (e r)")[:, ts(h, 128)],
                        rhs=id_sb,
                        is_transpose=True,
                    ).then_inc(s_tp, 1)

            def pe_mm2(j):
                if j == 0:
                    te.wait_ge(s_bc, 16)
                te.wait_ge(s_zwt, ERCH * j + 2)
                if j >= 1:                          # delta slot WAR: all of tile
                    te.wait_ge(s_epA, 2 * j)        # j-1's epilogues done
                    te.wait_ge(s_epV, 2 * j)
                for q in range(4):
                    for ch in range(ERCH):
                        mm = te.matmul(
                            d_ps[q],
                            lhsT=zwT_sb[:, j % 2, ch, :],
                            rhs=bc_sb[:, ch, ts(q, 512)],
                            start=(ch == 0),
                            stop=(ch == ERCH - 1),
                        )
                    mm.then_inc(s_d, 1)

            def emit_mm1(i):
                h = i // (NT // 2)          # token half this tile lives in
                for c in range(FCH):
                    if i in (0, NT // 2) and c % XG == 0:
                        if i == 0 and c == 0:
                            te.wait_ge(s_w, 16)
                        te.wait_ge(s_x[c // XG][h], 16)
                    if i == 0 and c == 1:
                        te.wait_ge(s_wr, 16)
                    # zs slot WAR vs abs/zw(i-3) is transitive: pe_transp(i-3)
                    # (earlier in PE order) waited s_zw >= i-2, and zw(i-3)
                    # implies abs(i-3) through DVE program order.
                    mm = te.matmul(
                        zs_ps[i % 2],
                        lhsT=x_sb[:, c // XG, h, c % XG, ts(i % (NT // 2), 128)],
                        rhs=w_sb[:, c, :],
                        start=(c == 0),
                        stop=(c == FCH - 1),
                    )
                mm.then_inc(s_mm1, 1)

            emit_mm1(0)
            emit_mm1(1)
            for i in range(NT):
                if i >= 1:
                    pe_mm2(i - 1)
                pe_transp(i)
                if i + 2 < NT:
                    emit_mm1(i + 2)
            pe_mm2(NT - 1)

        @block.scalar
        def _(act):
            def act_tail(j):
                for h in range(ERCH):
                    m = ERCH * j + h
                    act.wait_ge(s_tp, m + 1)
                    # zwT slot WAR vs mm2(j-2) covered: act_tail(j-1)'s ep
                    # waits reached s_d >= 4(j-1)+3 > 4(j-2)+4.
                    act.activation(
                        zwT_sb[:, j % 2, h, :], tp_ps[m % 2], AF.Copy
                    ).then_inc(s_zwt, 1)
                k, half = j // 2, j % 2
                for q in (0, 2):
                    act.wait_ge(s_d, 4 * j + q + 1)
                    if half == 0 and q == 0 and k >= 2:   # out_sb pair WAR
                        act.wait_ge([s_out0, s_out1][k % 2], 16 * (k // 2))
                    act.activation(
                        out_sb[:, k % 2, half, ts(q, 512)], d_ps[q],
                        AF.Copy,
                    ).then_inc(s_epA, 1)

            for i in range(NT):
                act.wait_ge(s_mm1, i + 1)
                act.activation(
                    sim_a[:, i, :], zs_ps[i % 2][:, ER:WCOLS], AF.Abs
                ).then_inc(s_abs, 1)
                act.drain()
                act.activation(
                    e_a[:, i, :], sim_a[:, i, :], AF.Exp
                ).then_inc(s_exp, 1)
                if i >= 1:
                    act_tail(i - 1)
            act_tail(NT - 1)

        @block.vector
        def _(ve):
            def dve_tail(j):
                k, half = j // 2, j % 2
                for q in (1, 3):
                    ve.wait_ge(s_d, 4 * j + q + 1)
                    if half == 0 and q == 1 and k >= 2:
                        ve.wait_ge([s_out0, s_out1][k % 2], 16 * (k // 2))
                    ve.tensor_copy(
                        out=out_sb[:, k % 2, half, ts(q, 512)], in_=d_ps[q]
                    ).then_inc(s_epV, 1)

            for i in range(NT):
                ve.wait_ge(s_abs, i + 1)
                ve.max(m8_a[:, i, :], sim_a[:, i, :])
                ve.drain()
                ve.tensor_scalar(
                    ge_a[:, i, :], sim_a[:, i, :], m8_a[:, i, 3:4], None,
                    op0=ALU.is_ge,
                )
                ve.wait_ge(s_exp, i + 1)
                ve.drain()
                ve.tensor_tensor(em_a[:, i, :], e_a[:, i, :], ge_a[:, i, :], ALU.mult)
                ve.drain()
                ve.tensor_reduce(
                    den_a[:, i:i + 1], em_a[:, i, :],
                    axis=mybir.AxisListType.X, op=ALU.add,
                )
                ve.drain()
                ve.reciprocal(rcp_a[:, i:i + 1], den_a[:, i:i + 1])
                ve.drain()
                ve.tensor_scalar(
                    co_a[:, i, :], em_a[:, i, :], rcp_a[:, i:i + 1], None,
                    op0=ALU.mult,
                )
                ve.drain()
                # zw slot WAR vs transp(i-2): covered — dve_tail(i-2) waited
                # s_d >= 4(i-2)+4, i.e. mm2(i-2) done, which on PE follows
                # transp(i-1).
                ve.tensor_tensor(
                    zw_sb[:, i % 2],
                    zs_ps[i % 2][:, 0:ER].rearrange("p (e r) -> p e r", r=R),
                    co_a[:, i, :, None].to_broadcast([128, E, R]),
                    ALU.mult,
                ).then_inc(s_zw, 1)
                if i >= 1:
                    dve_tail(i - 1)
            dve_tail(NT - 1)

    _sems.close()
    nc.finalize()
    return nc


def _host_prep(x, prototypes, A_stack, B_stack, scaling):
    tok = np.ascontiguousarray(x.reshape(T, F))

    Acat = A_stack.reshape(ER, F)
    Wh = np.concatenate([Acat.T, prototypes.T], axis=1)
    Wh = np.ascontiguousarray(
        Wh.reshape(FCH, 128, WCOLS).transpose(1, 0, 2)
    ).astype(np.float16)

    Bcat = (B_stack.transpose(0, 2, 1).reshape(ER, O) * float(scaling))
    Bch = Bcat.reshape(ERCH, 128, O).astype(np.float16)

    identh = np.eye(128, dtype=np.float16)

    in_maps = []
    for core in range(NCORES):
        shard = tok[core * TPC:(core + 1) * TPC]
        # (NG, 2, 128, XG, TPC//2): group, token-half, partition, chunk, tok
        xTh = (
            shard.T.reshape(NG, XG, 128, 2, TPC // 2)
            .transpose(0, 3, 2, 1, 4)
            .astype(np.float16)
        )
        in_maps.append({
            "xT": np.ascontiguousarray(xTh),
            "W": Wh,
            "Bc": Bch,
            "ident": identh,
        })
    return in_maps


def _setup_axon_tracing():
    import sys
    import types

    import concourse.bass_utils as bu

    bu.upload_artifacts = lambda tmpdir: "local://" + tmpdir
    try:
        from antenv.axon_hooks import get_axon_ntff_profile_hook  # noqa: F401
        return
    except ImportError:
        pass
    import antenv
    from trn_agent_boot.trn_boot import _ntff_profile_via_ctypes

    mod = types.ModuleType("antenv.axon_hooks")
    state = {"hook": _ntff_profile_via_ctypes("/opt/axon/libaxon_pjrt.so")}
    mod.set_axon_ntff_profile_hook = lambda h: state.__setitem__("hook", h)
    mod.get_axon_ntff_profile_hook = lambda: state["hook"]
    antenv.axon_hooks = mod
    sys.modules["antenv.axon_hooks"] = mod


def kernel(x, prototypes, A_stack, B_stack, scaling, top_k):
    global LAST_RESULTS
    assert int(top_k) == TOPK, f"kernel hardcodes top_k={TOPK}, got {top_k}"
    assert x.shape == (B, S, F)

    if "nc" not in _CACHE:
        _CACHE["nc"] = build_nc()
    nc = _CACHE["nc"]

    in_maps = _host_prep(
        np.asarray(x, dtype=np.float32),
        np.asarray(prototypes, dtype=np.float32),
        np.asarray(A_stack, dtype=np.float32),
        np.asarray(B_stack, dtype=np.float32),
        np.asarray(scaling, dtype=np.float32),
    )

    trace = os.environ.get("KERNEL_TRACE", "0") == "1"
    if trace:
        try:
            _setup_axon_tracing()
        except Exception as e:
            print(f"tracing setup failed ({e}); running without trace")
            trace = False
    res = run_bass_kernel_spmd(nc, in_maps, core_ids=list(range(NCORES)), trace=trace)
    LAST_RESULTS = res

    outs = [
        res.results[i]["out"].transpose(0, 2, 1, 3).reshape(TPC, O)
        for i in range(NCORES)
    ]
    full = np.concatenate(outs, axis=0).astype(np.float32)
    return full.reshape(B, S, O)


# revision 31
# speedup vs baseline: 1.1540x; 1.1540x over previous
"""ArrowLoRA MoE routing kernel for 8 TRN2 NeuronCores — raw Bass version.

Same math and host-side layout as the Tile version (see kernel.py), but
hand-written engine streams with explicit semaphores to eliminate the
Tile framework's ~7us start barrier, ~10us end drain, and scheduler
stalls.

Pipeline per core (1024 tokens = 8 tiles of 128):
  PE : mm1(i) [16 accum matmuls -> zs psum]  |  transp(i-1), mm2(i-1)
  ACT: |sim| -> exp (no max-shift; |sim|<=~6) ; zwT copies ; half epilogue
  DVE: top8 -> mask -> softmax -> coeff ; zw = z*coeff (reads PSUM) ; half epilogue
  SP : input DMAs (x groups)   GpSimd: W/Bc/ident DMAs + output DMAs
"""

import os

import numpy as np

import concourse.bass as bass
import concourse.mybir as mybir
from concourse import bacc
from concourse.bass import ts
from concourse.bass_utils import run_bass_kernel_spmd

B, S, F, E, R, O = 4, 2048, 2048, 16, 16, 2048
TOPK = 4
NCORES = 8
T = B * S
TPC = T // NCORES          # 1024
NT = TPC // 128            # 8
FCH = F // 128             # 16
ER = E * R                 # 256
ERCH = ER // 128           # 2
WCOLS = ER + E             # 272
XG = 4
NG = FCH // XG

F16 = mybir.dt.float16
F32 = mybir.dt.float32
ALU = mybir.AluOpType
AF = mybir.ActivationFunctionType

_CACHE = {}
LAST_RESULTS = None


def build_nc(warm_n=0, w_split=False, micro=False, fine_flush=False):
    nc = bacc.Bacc(target_bir_lowering=False)

    xT = nc.declare_dram_parameter("xT", [NG, 2, 128, XG, TPC // 2], F16, isOutput=False)
    W = nc.declare_dram_parameter("W", [128, FCH, WCOLS], F16, isOutput=False)
    Bc = nc.declare_dram_parameter("Bc", [ERCH, 128, O], F16, isOutput=False)
    ident = nc.declare_dram_parameter("ident", [128, 128], F16, isOutput=False)
    out = nc.declare_dram_parameter("out", [NT // 2, 128, 2, O], F16, isOutput=True)

    # ---- SBUF ----
    x_sb = nc.alloc_sbuf_tensor("x_sb", [128, NG, 2, XG, TPC // 2], F16).ap()
    w_sb = nc.alloc_sbuf_tensor("w_sb", [128, FCH, WCOLS], F16).ap()
    bc_sb = nc.alloc_sbuf_tensor("bc_sb", [128, ERCH, O], F16).ap()
    id_sb = nc.alloc_sbuf_tensor("id_sb", [128, 128], F16).ap()
    sim_a = nc.alloc_sbuf_tensor("sim_a", [128, NT, E], F32).ap()
    m8_a = nc.alloc_sbuf_tensor("m8_a", [128, NT, 8], F32).ap()
    e_a = nc.alloc_sbuf_tensor("e_a", [128, NT, E], F32).ap()
    ge_a = nc.alloc_sbuf_tensor("ge_a", [128, NT, E], F32).ap()
    em_a = nc.alloc_sbuf_tensor("em_a", [128, NT, E], F32).ap()
    den_a = nc.alloc_sbuf_tensor("den_a", [128, NT], F32).ap()
    rcp_a = nc.alloc_sbuf_tensor("rcp_a", [128, NT], F32).ap()
    co_a = nc.alloc_sbuf_tensor("co_a", [128, NT, E], F32).ap()
    zw_sb = nc.alloc_sbuf_tensor("zw_sb", [128, 2, E, R], F16).ap()      # 2 bufs
    zwT_sb = nc.alloc_sbuf_tensor("zwT_sb", [128, 2, ERCH, 128], F16).ap()
    out_sb = nc.alloc_sbuf_tensor("out_sb", [128, 2, 2, O], F16).ap()    # 2 pairs
    warm_sb = nc.alloc_sbuf_tensor("warm_sb", [128, 512], F16).ap() if warm_n else None

    # ---- PSUM: 8 banks = zs x2 + tp x2 + delta x4 ----
    zs_ps = [nc.alloc_psum_tensor(f"zs{s}", [128, WCOLS], F32).ap() for s in range(2)]
    tp_ps = [nc.alloc_psum_tensor(f"tp{s}", [128, 128], F16).ap() for s in range(2)]
    d_ps = [nc.alloc_psum_tensor(f"d{q}", [128, 512], F32).ap() for q in range(4)]

    from contextlib import ExitStack
    _sems = ExitStack()

    def sem(name):
        return _sems.enter_context(nc.semaphore(name))

    s_w, s_wr = sem("s_w"), sem("s_wr")
    s_wg2, s_wg3 = sem("s_wg2"), sem("s_wg3")
    s_x00, s_x10, s_x20, s_x30 = sem("s_x00"), sem("s_x10"), sem("s_x20"), sem("s_x30")
    s_x01, s_x11, s_x21, s_x31 = sem("s_x01"), sem("s_x11"), sem("s_x21"), sem("s_x31")
    s_id, s_bc = sem("s_id"), sem("s_bc")
    s_mm1, s_abs, s_exp, s_zw = sem("s_mm1"), sem("s_abs"), sem("s_exp"), sem("s_zw")
    s_tp, s_zwt, s_d = sem("s_tp"), sem("s_zwt"), sem("s_d")
    s_epA, s_epV = sem("s_epA"), sem("s_epV")
    s_out0, s_out1 = sem("s_out0"), sem("s_out1")
    s_warm = sem("s_warm") if warm_n else None

    with nc.Block(no_gpsimd_drain=True) as block:
        s_x = [[s_x00, s_x01], [s_x10, s_x11], [s_x20, s_x21], [s_x30, s_x31]]

        @block.sync
        def _(sp):
            if w_split:
                # Interleave W chunk-groups with the x groups so the bulk of
                # W never delays the x group that the matmul pipeline is
                # about to need.
                s_wg = [s_w, s_wr, s_wg2, s_wg3]
                sp.dma_start(out=w_sb[:, 0:XG, :], in_=W[:, 0:XG, :]).then_inc(s_wg[0], 16)
                sp.dma_start(out=x_sb[:, 0, 0], in_=xT[0, 0]).then_inc(s_x[0][0], 16)
                for g in range(1, NG):
                    sp.dma_start(out=x_sb[:, g, 0], in_=xT[g, 0]).then_inc(s_x[g][0], 16)
                    sp.dma_start(
                        out=w_sb[:, XG * g:XG * (g + 1), :],
                        in_=W[:, XG * g:XG * (g + 1), :],
                    ).then_inc(s_wg[g], 16)
                for g in range(NG):
                    sp.dma_start(out=x_sb[:, g, 1], in_=xT[g, 1]).then_inc(s_x[g][1], 16)
            else:
                # Priority order: W chunk 0 (70KB), x(group0, token-half0),
                # rest of W, remaining half-0 groups, then half-1 groups.
                sp.dma_start(out=w_sb[:, 0:1, :], in_=W[:, 0:1, :]).then_inc(s_w, 16)
                sp.dma_start(out=x_sb[:, 0, 0], in_=xT[0, 0]).then_inc(s_x[0][0], 16)
                sp.dma_start(out=w_sb[:, 1:FCH, :], in_=W[:, 1:FCH, :]).then_inc(s_wr, 16)
                for g in range(1, NG):
                    sp.dma_start(out=x_sb[:, g, 0], in_=xT[g, 0]).then_inc(s_x[g][0], 16)
                for g in range(NG):
                    sp.dma_start(out=x_sb[:, g, 1], in_=xT[g, 1]).then_inc(s_x[g][1], 16)

        @block.gpsimd
        def _(gp):
            # gpsimd starts ~6us late (library load) - only late-needed DMAs.
            # Wait for x0 so bc/ident descriptors queue behind the
            # critical-path x groups, not ahead of them.
            gp.wait_ge(s_x00, 16)
            gp.dma_start(
                out=bc_sb, in_=Bc[:, :, :].rearrange("c p o -> p c o")
            ).then_inc(s_bc, 16)
            gp.dma_start(out=id_sb, in_=ident[:, :]).then_inc(s_id, 16)
            s_out = [s_out0, s_out1]
            for k in range(NT // 2 - 1):
                gp.wait_ge(s_epA, 4 * (k + 1))
                gp.wait_ge(s_epV, 4 * (k + 1))
                gp.dma_start(out=out[k], in_=out_sb[:, k % 2]).then_inc(s_out[k % 2], 16)
            kl = NT // 2 - 1
            gp.wait_ge(s_epA, 4 * kl + 2)
            gp.wait_ge(s_epV, 4 * kl + 2)
            gp.dma_start(
                out=out[kl][:, 0:1, :], in_=out_sb[:, kl % 2, 0:1, :]
            ).then_inc(s_out[kl % 2], 16)
            if micro or fine_flush:
                # last tile in two pieces as its epilogue quarters land
                gp.wait_ge(s_epA, 4 * kl + 3)
                gp.wait_ge(s_epV, 4 * kl + 3)
                gp.dma_start(
                    out=out[kl][:, 1:2, 0:1024],
                    in_=out_sb[:, kl % 2, 1:2, 0:1024],
                ).then_inc(s_out[kl % 2], 16)
                gp.wait_ge(s_epA, 4 * kl + 4)
                gp.wait_ge(s_epV, 4 * kl + 4)
                gp.dma_start(
                    out=out[kl][:, 1:2, 1024:2048],
                    in_=out_sb[:, kl % 2, 1:2, 1024:2048],
                ).then_inc(s_out[kl % 2], 16)
                gp.wait_ge(s_out0, 32)
                gp.wait_ge(s_out1, 64)
            else:
                gp.wait_ge(s_epA, 4 * kl + 4)
                gp.wait_ge(s_epV, 4 * kl + 4)
                gp.dma_start(
                    out=out[kl][:, 1:2, :], in_=out_sb[:, kl % 2, 1:2, :]
                ).then_inc(s_out[kl % 2], 16)
                gp.wait_ge(s_out0, 32)
                gp.wait_ge(s_out1, 48)

        @block.tensor
        def _(te):
            def pe_transp(j):
                te.wait_ge(s_zw, j + 1)
                if j == 0:
                    te.wait_ge(s_id, 16)
                for h in range(ERCH):
                    m = ERCH * j + h
                    # tp slot WAR vs ACT copy(m-2) is covered transitively:
                    # pe_mm2(j-1) already waited s_zwt >= 2j >= m-1.
                    te.matmul(
                        tp_ps[m % 2],
                        lhsT=zw_sb[:, j % 2].rearrange("p e r -> p (e r)")[:, ts(h, 128)],
                        rhs=id_sb,
                        is_transpose=True,
                    ).then_inc(s_tp, 1)

            def pe_mm2(j):
                if j == 0:
                    te.wait_ge(s_bc, 16)
                if j >= 1:                          # delta slot WAR: all of tile
                    te.wait_ge(s_epA, 2 * j)        # j-1's epilogues done
                    te.wait_ge(s_epV, 2 * j)
                if micro:
                    # chunk-0 matmuls only need the first zwT copy; chunk-1
                    # waits for the second. Quarter accumulation groups stay
                    # valid (start on ch0, stop on ch1, distinct banks).
                    te.wait_ge(s_zwt, ERCH * j + 1)
                    for q in range(4):
                        te.matmul(
                            d_ps[q],
                            lhsT=zwT_sb[:, j % 2, 0, :],
                            rhs=bc_sb[:, 0, ts(q, 512)],
                            start=True, stop=False,
                        )
                    te.wait_ge(s_zwt, ERCH * j + 2)
                    for q in range(4):
                        te.matmul(
                            d_ps[q],
                            lhsT=zwT_sb[:, j % 2, 1, :],
                            rhs=bc_sb[:, 1, ts(q, 512)],
                            start=False, stop=True,
                        ).then_inc(s_d, 1)
                else:
                    te.wait_ge(s_zwt, ERCH * j + 2)
                    for q in range(4):
                        for ch in range(ERCH):
                            mm = te.matmul(
                                d_ps[q],
                                lhsT=zwT_sb[:, j % 2, ch, :],
                                rhs=bc_sb[:, ch, ts(q, 512)],
                                start=(ch == 0),
                                stop=(ch == ERCH - 1),
                            )
                        mm.then_inc(s_d, 1)

            if warm_n:
                # dummy matmuls spanning the pre-input window: unthrottle the
                # PE HAM clock gate so real matmuls start at 2.4GHz
                te.wait_ge(s_warm, 1)
                for _ in range(warm_n):
                    te.matmul(d_ps[0], lhsT=warm_sb[:, 0:128], rhs=warm_sb,
                              start=True, stop=True)

            def emit_mm1(i):
                h = i // (NT // 2)          # token half this tile lives in
                for c in range(FCH):
                    if i in (0, NT // 2) and c % XG == 0:
                        if i == 0 and c == 0:
                            te.wait_ge(s_w, 16)
                        if w_split and i == 0:
                            te.wait_ge([s_w, s_wr, s_wg2, s_wg3][c // XG], 16)
                        te.wait_ge(s_x[c // XG][h], 16)
                    if (not w_split) and i == 0 and c == 1:
                        te.wait_ge(s_wr, 16)
                    # zs slot WAR vs abs/zw(i-3) is transitive: pe_transp(i-3)
                    # (earlier in PE order) waited s_zw >= i-2, and zw(i-3)
                    # implies abs(i-3) through DVE program order.
                    mm = te.matmul(
                        zs_ps[i % 2],
                        lhsT=x_sb[:, c // XG, h, c % XG, ts(i % (NT // 2), 128)],
                        rhs=w_sb[:, c, :],
                        start=(c == 0),
                        stop=(c == FCH - 1),
                    )
                mm.then_inc(s_mm1, 1)

            emit_mm1(0)
            emit_mm1(1)
            for i in range(NT):
                if i >= 1:
                    pe_mm2(i - 1)
                pe_transp(i)
                if i + 2 < NT:
                    emit_mm1(i + 2)
            pe_mm2(NT - 1)

        @block.scalar
        def _(act):
            def act_tail(j):
                for h in range(ERCH):
                    m = ERCH * j + h
                    act.wait_ge(s_tp, m + 1)
                    # zwT slot WAR vs mm2(j-2) covered: act_tail(j-1)'s ep
                    # waits reached s_d >= 4(j-1)+3 > 4(j-2)+4.
                    act.activation(
                        zwT_sb[:, j % 2, h, :], tp_ps[m % 2], AF.Copy
                    ).then_inc(s_zwt, 1)
                k, half = j // 2, j % 2
                for q in (0, 2):
                    act.wait_ge(s_d, 4 * j + q + 1)
                    if half == 0 and q == 0 and k >= 2:   # out_sb pair WAR
                        act.wait_ge([s_out0, s_out1][k % 2], 16 * (k // 2))
                    act.activation(
                        out_sb[:, k % 2, half, ts(q, 512)], d_ps[q],
                        AF.Copy,
                    ).then_inc(s_epA, 1)

            for i in range(NT):
                act.wait_ge(s_mm1, i + 1)
                act.activation(
                    sim_a[:, i, :], zs_ps[i % 2][:, ER:WCOLS], AF.Abs
                ).then_inc(s_abs, 1)
                act.drain()
                act.activation(
                    e_a[:, i, :], sim_a[:, i, :], AF.Exp
                ).then_inc(s_exp, 1)
                if i >= 1:
                    act_tail(i - 1)
            act_tail(NT - 1)

        @block.vector
        def _(ve):
            def dve_tail(j):
                k, half = j // 2, j % 2
                for q in (1, 3):
                    ve.wait_ge(s_d, 4 * j + q + 1)
                    if half == 0 and q == 1 and k >= 2:
                        ve.wait_ge([s_out0, s_out1][k % 2], 16 * (k // 2))
                    ve.tensor_copy(
                        out=out_sb[:, k % 2, half, ts(q, 512)], in_=d_ps[q]
                    ).then_inc(s_epV, 1)

            if warm_n:
                ve.memset(warm_sb, 0.0).then_inc(s_warm, 1)
            for i in range(NT):
                ve.wait_ge(s_abs, i + 1)
                ve.max(m8_a[:, i, :], sim_a[:, i, :])
                ve.drain()
                ve.tensor_scalar(
                    ge_a[:, i, :], sim_a[:, i, :], m8_a[:, i, 3:4], None,
                    op0=ALU.is_ge,
                )
                ve.wait_ge(s_exp, i + 1)
                ve.drain()
                ve.tensor_tensor(em_a[:, i, :], e_a[:, i, :], ge_a[:, i, :], ALU.mult)
                ve.drain()
                ve.tensor_reduce(
                    den_a[:, i:i + 1], em_a[:, i, :],
                    axis=mybir.AxisListType.X, op=ALU.add,
                )
                ve.drain()
                ve.reciprocal(rcp_a[:, i:i + 1], den_a[:, i:i + 1])
                ve.drain()
                ve.tensor_scalar(
                    co_a[:, i, :], em_a[:, i, :], rcp_a[:, i:i + 1], None,
                    op0=ALU.mult,
                )
                ve.drain()
                # zw slot WAR vs transp(i-2): covered — dve_tail(i-2) waited
                # s_d >= 4(i-2)+4, i.e. mm2(i-2) done, which on PE follows
                # transp(i-1).
                ve.tensor_tensor(
                    zw_sb[:, i % 2],
                    zs_ps[i % 2][:, 0:ER].rearrange("p (e r) -> p e r", r=R),
                    co_a[:, i, :, None].to_broadcast([128, E, R]),
                    ALU.mult,
                ).then_inc(s_zw, 1)
                if i >= 1:
                    dve_tail(i - 1)
            dve_tail(NT - 1)

    _sems.close()
    nc.finalize()
    return nc


def _host_prep(x, prototypes, A_stack, B_stack, scaling):
    tok = np.ascontiguousarray(x.reshape(T, F))

    Acat = A_stack.reshape(ER, F)
    Wh = np.concatenate([Acat.T, prototypes.T], axis=1)
    Wh = np.ascontiguousarray(
        Wh.reshape(FCH, 128, WCOLS).transpose(1, 0, 2)
    ).astype(np.float16)

    Bcat = (B_stack.transpose(0, 2, 1).reshape(ER, O) * float(scaling))
    Bch = Bcat.reshape(ERCH, 128, O).astype(np.float16)

    identh = np.eye(128, dtype=np.float16)

    in_maps = []
    for core in range(NCORES):
        shard = tok[core * TPC:(core + 1) * TPC]
        # (NG, 2, 128, XG, TPC//2): group, token-half, partition, chunk, tok
        xTh = (
            shard.T.reshape(NG, XG, 128, 2, TPC // 2)
            .transpose(0, 3, 2, 1, 4)
            .astype(np.float16)
        )
        in_maps.append({
            "xT": np.ascontiguousarray(xTh),
            "W": Wh,
            "Bc": Bch,
            "ident": identh,
        })
    return in_maps


def _setup_axon_tracing():
    import sys
    import types

    import concourse.bass_utils as bu

    bu.upload_artifacts = lambda tmpdir: "local://" + tmpdir
    try:
        from antenv.axon_hooks import get_axon_ntff_profile_hook  # noqa: F401
        return
    except ImportError:
        pass
    import antenv
    from trn_agent_boot.trn_boot import _ntff_profile_via_ctypes

    mod = types.ModuleType("antenv.axon_hooks")
    state = {"hook": _ntff_profile_via_ctypes("/opt/axon/libaxon_pjrt.so")}
    mod.set_axon_ntff_profile_hook = lambda h: state.__setitem__("hook", h)
    mod.get_axon_ntff_profile_hook = lambda: state["hook"]
    antenv.axon_hooks = mod
    sys.modules["antenv.axon_hooks"] = mod


def kernel(x, prototypes, A_stack, B_stack, scaling, top_k):
    global LAST_RESULTS
    assert int(top_k) == TOPK, f"kernel hardcodes top_k={TOPK}, got {top_k}"
    assert x.shape == (B, S, F)

    if "nc" not in _CACHE:
        _CACHE["nc"] = build_nc(fine_flush=True)
    nc = _CACHE["nc"]

    in_maps = _host_prep(
        np.asarray(x, dtype=np.float32),
        np.asarray(prototypes, dtype=np.float32),
        np.asarray(A_stack, dtype=np.float32),
        np.asarray(B_stack, dtype=np.float32),
        np.asarray(scaling, dtype=np.float32),
    )

    trace = os.environ.get("KERNEL_TRACE", "0") == "1"
    if trace:
        try:
            _setup_axon_tracing()
        except Exception as e:
            print(f"tracing setup failed ({e}); running without trace")
            trace = False
    res = run_bass_kernel_spmd(nc, in_maps, core_ids=list(range(NCORES)), trace=trace)
    LAST_RESULTS = res

    outs = [
        res.results[i]["out"].transpose(0, 2, 1, 3).reshape(TPC, O)
        for i in range(NCORES)
    ]
    full = np.concatenate(outs, axis=0).astype(np.float32)
    return full.reshape(B, S, O)
